# revision 16
# baseline (speedup 1.0000x reference)
"""Trainium2 distributed kernel for nn_ARDecoder (2x1024 tokens, D=1024,
H=16/KV=4 GQA, DFF=4096, V=32000, P=256, 4 layers).

Strategy: data-parallel over the 2048 (batch*seq) rows -- 256 rows per core,
assigned as the interleaved 128-row blocks {q, 7-q} (q = core%4) of its
batch so causal attention is load-balanced: every core scores its rt0 rows
against key blocks 0..3 and its rt1 rows against key blocks 0..7 (12 block
columns instead of 16), with the causal structure expressed purely in
per-core host-built ADDITIVE masks (0 / -1e9) that are accumulated into the
score PSUM by identity matmuls on the PE, so softmax is a single exp on the
scalar engine with no separate masked copy. Weights are replicated (bf16),
activations stay SBUF-resident. K^T and V are all-gathered per batch group
([[0..3],[4..7]]); both communicators are warmed by dummy gathers at program
start so launch skew is absorbed before layer 0. The logits GEMM is
vocab-sharded: h@patch_W.T is all-gathered in two row-halves (the second
overlaps the first half's logits GEMM) and each core computes its
4000-column slice of E. The MLP keeps its intermediate transposed
(f-major) so no PE transposes are needed between Wg/Wu and Wd.
"""

import os
import numpy as np
import ml_dtypes

import concourse.bass as bass
import concourse.bacc as bacc
import concourse.mybir as mybir
import concourse.tile as tile
from concourse.bass_utils import run_bass_kernel_spmd
from concourse.masks import make_identity

BF16 = mybir.dt.bfloat16
F32 = mybir.dt.float32
F8 = mybir.dt.float8e4
DR = mybir.MatmulPerfMode.DoubleRow
SF, SG, SU, SD, SM = 16.0, 512.0, 512.0, 1024.0, 16.0
AF = mybir.ActivationFunctionType
AL = mybir.AluOpType

N_CORES = 8
CORE_IDS = list(range(N_CORES))
B, T, D, H, KV, HD, DFF, V, P, DLAT, L = 2, 1024, 1024, 16, 4, 64, 4096, 32000, 256, 512, 4
EPS = 1e-6
R = 256            # rows per core
RT = 2             # row tiles of 128
DKT = D // 128     # 8 k-tiles over D
VSH = V // N_CORES # 4000 vocab columns per core
NKT = 8            # key tiles of 128 within a batch

_cache = {}


def build(dbg=False):
    key = ("nc", dbg)
    if key in _cache:
        return _cache[key]
    nc = bacc.Bacc("TRN2", target_bir_lowering=False, debug=False,
                   num_devices=N_CORES)
    dbg_t = {}
    if dbg:
        for name, shape, dt in [
            ("dbg_h0", [128, RT, D], F32), ("dbg_h1", [128, RT, D], F32),
            ("dbg_h2", [128, RT, D], F32), ("dbg_h3", [128, RT, D], F32),
            ("dbg_h4", [128, RT, D], F32),
            ("dbg_qT", [128, DKT, R], BF16), ("dbg_kT", [128, KV, T], BF16),
            ("dbg_v65", [128, NKT, KV, HD + 1], BF16),
            ("dbg_oT", [128, 8, R], BF16), ("dbg_hp", [128, 2, R], BF16),
            ("dbg_er", [128, NKT, 2, 2, R], BF16),
        ]:
            dbg_t[name] = nc.dram_tensor(name, shape, dt, kind="ExternalOutput")

    # ---- parameters (per-core inputs) ----
    ex_augT = nc.dram_tensor("ex_augT", [128, 3, R], BF16, kind="ExternalInput")
    w_emb = nc.dram_tensor("w_emb", [128, 3, D], BF16, kind="ExternalInput")
    wqT = nc.dram_tensor("wqT", [L, 128, DKT, D], BF16, kind="ExternalInput")
    wkT = nc.dram_tensor("wkT", [L, 128, DKT, KV * HD], BF16, kind="ExternalInput")
    wvT = nc.dram_tensor("wvT", [L, 128, DKT, KV * HD], BF16, kind="ExternalInput")
    woT = nc.dram_tensor("woT", [L, 128, DKT, D], BF16, kind="ExternalInput")
    wgT = nc.dram_tensor("wgT", [L, 128, 4, DKT, 1024], BF16, kind="ExternalInput")
    wuT = nc.dram_tensor("wuT", [L, 128, 4, DKT, 1024], BF16, kind="ExternalInput")
    wdT = nc.dram_tensor("wdT", [L, 128, 4, DKT, D], BF16, kind="ExternalInput")
    n1rep = nc.dram_tensor("n1rep", [L, 128, D], F32, kind="ExternalInput")
    # additive mask slots 0-3: rt0 half vs key blocks 0-3; 4-7: rt1 vs 4-7
    mask2 = nc.dram_tensor("mask2", [128, NKT, 128], BF16, kind="ExternalInput")
    patchT = nc.dram_tensor("patchT", [D, P], BF16, kind="ExternalInput")
    ecT = nc.dram_tensor("ecT", [P, VSH], BF16, kind="ExternalInput")
    out = nc.dram_tensor("logits", [B * T, VSH], BF16, kind="ExternalOutput")

    # ---- internal DRAM (collective bounce buffers) ----
    k_in, k_g, v_in, v_g = [], [], [], []
    for l in range(L):
        k_in.append(nc.dram_tensor(f"k_in{l}", [64, KV, R], BF16))
        k_g.append(nc.dram_tensor(f"k_g{l}", [4, 64, KV, R], BF16))
        v_in.append(nc.dram_tensor(f"v_in{l}", [128, RT, KV, HD], BF16))
        v_g.append(nc.dram_tensor(f"v_g{l}", [4, 128, RT, KV, HD], BF16))
    hp_in = [nc.dram_tensor(f"hp_in{s}", [128, 2, 128], BF16) for s in range(2)]
    hp_g = [nc.dram_tensor(f"hp_g{s}", [8, 128, 2, 128], BF16,
                           addr_space="Shared") for s in range(2)]
    dum_in = nc.dram_tensor("dum_in", [128], BF16)
    dum_g = nc.dram_tensor("dum_g", [8, 128], BF16)
    dum2_in = nc.dram_tensor("dum2_in", [128], BF16)
    dum2_g = nc.dram_tensor("dum2_g", [4, 128], BF16)

    GROUPS_KV = [[0, 1, 2, 3], [4, 5, 6, 7]]
    GROUPS_ALL = [CORE_IDS]

    with tile.TileContext(nc) as tc:
        with (
            tc.tile_pool(name="const", bufs=1) as cpool,
            tc.tile_pool(name="persist", bufs=1) as pp,
            tc.tile_pool(name="wts", bufs=2) as wp,
            tc.tile_pool(name="acts", bufs=1) as ap,
        ):
            # warm BOTH communicators with dummy gathers before any compute
            # so launch skew + communicator init overlap the embed phase
            # instead of stalling layer-0's K gather.
            dum_sb = cpool.tile([1, 128], BF16)
            nc.vector.memset(dum_sb[:], 0.0)
            nc.sync.dma_start(dum_in[:], dum_sb[:])

            ident = cpool.tile([128, 128], BF16)
            make_identity(nc, ident[:])
            epsb = cpool.tile([128, 1], F32)
            nc.vector.memset(epsb[:], EPS)

            h = pp.tile([128, RT, D], F32)
            mask_sb = pp.tile([128, NKT, 128], BF16)

            def rms_scales(src, tag):
                """Returns (r, s) tiles [128, RT] f32: r = 1/sqrt(ms+eps),
                s = sqrt(ms+eps)."""
                ss = ap.tile([128, RT], F32, name=f"ss_{tag}", tag=f"ss_{tag}")
                for rt in range(RT):
                    sq = ap.tile([128, D], F32, name=f"sq_{tag}{rt}",
                                 tag="sq_scratch", bufs=1)
                    nc.scalar.activation(sq[:], src[:, rt, :], AF.Square,
                                         accum_out=ss[:, rt : rt + 1])
                sg = ap.tile([128, RT], F32, name=f"sg_{tag}", tag=f"sg_{tag}")
                rr = ap.tile([128, RT], F32, name=f"rr_{tag}", tag=f"rr_{tag}")
                for rt in range(RT):  # per-rt so downstream rt0 work can start
                    nc.scalar.activation(sg[:, rt : rt + 1], ss[:, rt : rt + 1],
                                         AF.Sqrt, scale=1.0 / D, bias=epsb[:])
                    nc.vector.reciprocal_approx_fast(rr[:, rt : rt + 1],
                                                     sg[:, rt : rt + 1])
                return rr, sg

            def transpose_to(dst_ap, src_ap, psum_pool, tag):
                """PE-transpose one [128,128] bf16 tile src_ap -> dst_ap."""
                tp = psum_pool.tile([128, 128], BF16, name=f"tp_{tag}", tag="tp",
                                    bufs=2)
                nc.tensor.transpose(tp[:], src_ap, ident[:])
                nc.vector.tensor_copy(dst_ap, tp[:])

            # ================= embedding =================
            exT = ap.tile([128, 3, R], BF16)
            wem = ap.tile([128, 3, D], BF16)
            nc.sync.dma_start(exT[:], ex_augT[:])
            nc.sync.dma_start(wem[:], w_emb[:])
            nc.sync.dma_start(mask_sb[:], mask2[:])
            with tc.tile_pool(name="ps_emb", bufs=4, space="PSUM") as pse:
                for rt in range(RT):
                    for nch in range(2):
                        psum = pse.tile([128, 512], F32, tag="mm")
                        for j in range(3):
                            nc.tensor.matmul(
                                psum[:],
                                exT[:, j, rt * 128:(rt + 1) * 128],
                                wem[:, j, nch * 512:(nch + 1) * 512],
                                start=(j == 0), stop=(j == 2))
                        nc.scalar.copy(h[:, rt, nch * 512:(nch + 1) * 512], psum[:])

            if dbg:
                nc.sync.dma_start(dbg_t["dbg_h0"][:], h[:])
            # ================= layers =================
            for l in range(L):
                # transpose psum pool spans the whole layer (2 banks) so
                # PE transposes interleave with the adjacent matmul phases
                ptp = tc.alloc_tile_pool(name=f"ps_tp{l}", bufs=2, space="PSUM")
                # ---- norm1 + double-rms for q/k ----
                n1 = wp.tile([128, D], F32, tag="n1")
                nc.sync.dma_start(n1[:], n1rep[l])
                r1, _ = rms_scales(h, f"n1_{l}")
                # hw = h*norm1_w; ms(hn) = r1^2 * ms(hw), so the second rms
                # scale folds into the sqrt: qkb = hw * (r1*r2)
                hw = ap.tile([128, RT, D], F32, name=f"hw{l}", tag="hw_f32")
                ss2 = ap.tile([128, RT], F32, name=f"ss2_{l}", tag="ss2")
                for rt in range(RT):
                    nc.vector.tensor_tensor(hw[:, rt, :], h[:, rt, :], n1[:],
                                            mybir.AluOpType.mult)
                    sq2 = ap.tile([128, D], F32, name=f"sq2_{l}{rt}",
                                  tag="sq_scratch", bufs=1)
                    nc.scalar.activation(sq2[:], hw[:, rt, :], AF.Square,
                                         accum_out=ss2[:, rt : rt + 1])
                r1d = ap.tile([128, RT], F32, name=f"r1d{l}", tag="r1d")
                nc.vector.tensor_tensor(r1d[:], r1[:], r1[:], mybir.AluOpType.mult)
                nc.vector.tensor_scalar(r1d[:], r1d[:], 1.0 / D, None,
                                        mybir.AluOpType.mult)
                s2 = ap.tile([128, RT], F32, name=f"s2_{l}", tag="s2t")
                r12 = ap.tile([128, RT], F32, name=f"r12_{l}", tag="r12")
                qkb = ap.tile([128, RT, D], BF16, tag="qkb")
                for rt in range(RT):
                    nc.scalar.activation(s2[:, rt : rt + 1], ss2[:, rt : rt + 1],
                                         AF.Sqrt, scale=r1d[:, rt : rt + 1],
                                         bias=epsb[:])
                    r2s = ap.tile([128, 1], F32, name=f"r2s{l}{rt}", tag="r2s",
                                  bufs=2)
                    nc.vector.reciprocal_approx_fast(r2s[:], s2[:, rt : rt + 1])
                    nc.vector.tensor_tensor(r12[:, rt : rt + 1],
                                            r1[:, rt : rt + 1], r2s[:],
                                            mybir.AluOpType.mult)
                    nc.vector.tensor_scalar(qkb[:, rt, :], hw[:, rt, :],
                                            r12[:, rt : rt + 1], None,
                                            mybir.AluOpType.mult)
                qkT = ap.tile([128, DKT, R], BF16, tag="xT")
                for kt in range(DKT):  # kt-outer: K matmul kt can start early
                    for rt in range(RT):
                        transpose_to(qkT[:, kt, rt * 128:(rt + 1) * 128],
                                     qkb[:, rt, kt * 128:(kt + 1) * 128],
                                     ptp, f"qk{l}")
                ptp.release()

                # ---- K, V first (kick the all-gather early), then Q ----
                qT = ap.tile([128, DKT, R], BF16, tag="qT")
                k_stage = ap.tile([128, 2, R], BF16, tag="k_stage")
                v_stage = ap.tile([128, RT, KV, HD], BF16, tag="v_stage")
                wk_sb = ap.tile([128, DKT, KV * HD], BF16, name=f"wk{l}", tag="wk_sb")
                wv_sb = ap.tile([128, DKT, KV * HD], BF16, name=f"wv{l}", tag="wv_sb")
                nc.sync.dma_start(wk_sb[:], wkT[l])
                nc.sync.dma_start(wv_sb[:], wvT[l])
                with tc.tile_pool(name=f"ps_qkv{l}", bufs=5, space="PSUM") as pq:
                    for mt in range(2):    # k^T [kvdim, rows]
                        psk = pq.tile([128, R], F32, tag="mm")
                        for kt in range(DKT):
                            nc.tensor.matmul(psk[:],
                                             wk_sb[:, kt, mt * 128:(mt + 1) * 128],
                                             qkT[:, kt, :],
                                             start=(kt == 0), stop=(kt == DKT - 1))
                        nc.scalar.copy(k_stage[:, mt, :], psk[:])
                    for gp in range(2):  # g in {gp, gp+2} share partitions
                        nc.sync.dma_start(
                            k_in[l][:, gp::2, :],
                            k_stage[64 * gp:64 * gp + 64, :, :])
                    nc.gpsimd.collective_compute(
                        "AllGather", mybir.AluOpType.bypass,
                        replica_groups=GROUPS_KV,
                        ins=[k_in[l][:].opt()], outs=[k_g[l][:].opt()])
                    for rt in range(RT):   # v [rows, kvdim] (undo r2 via s2)
                        psv = pq.tile([128, KV * HD], F32, tag="mm")
                        for kt in range(DKT):
                            nc.tensor.matmul(psv[:],
                                             qkT[:, kt, rt * 128:(rt + 1) * 128],
                                             wv_sb[:, kt, :],
                                             start=(kt == 0), stop=(kt == DKT - 1))
                        nc.vector.tensor_scalar(
                            v_stage[:, rt, :, :],
                            psv[:].rearrange("p (a b) -> p a b", a=KV),
                            s2[:, rt : rt + 1], None, mybir.AluOpType.mult)
                    nc.sync.dma_start(v_in[l][:], v_stage[:])
                    nc.gpsimd.collective_compute(
                        "AllGather", mybir.AluOpType.bypass,
                        replica_groups=GROUPS_KV,
                        ins=[v_in[l][:].opt()], outs=[v_g[l][:].opt()])
                    if l == 0:
                        # warm the all-8 communicator (for the logits
                        # gathers) while the CC queue is otherwise idle
                        nc.gpsimd.collective_compute(
                            "AllGather", mybir.AluOpType.bypass,
                            replica_groups=GROUPS_ALL,
                            ins=[dum_in[:].opt()], outs=[dum_g[:].opt()])
                    # Q while the gathers are in flight
                    wqb = ap.tile([128, DKT, D], BF16, name=f"wq{l}", tag="wbig",
                                  bufs=3)
                    nc.sync.dma_start(wqb[:], wqT[l])
                    for mt in range(DKT):  # q^T [qdim, rows]
                        psq = pq.tile([128, R], F32, tag="mm")
                        for kt in range(DKT):
                            nc.tensor.matmul(psq[:],
                                             wqb[:, kt, mt * 128:(mt + 1) * 128],
                                             qkT[:, kt, :],
                                             start=(kt == 0), stop=(kt == DKT - 1))
                        nc.scalar.copy(qT[:, mt, :], psq[:])

                # gathered chunk b holds core b's rows = global blocks {b, 7-b}
                kT_both = ap.tile([128, KV, T], BF16, tag="kT_both")
                v65 = ap.tile([128, NKT, KV, HD + 1], BF16, tag="v65")
                nc.vector.memset(v65[:], 1.0)
                # batched loads in gather-chunk order: chunk b lands at slot b
                # (its rt0-range block b) and slot 4+b (its rt1-range block
                # 7-b); host masks are permuted to match the slot order.
                for half in range(2):
                    p0 = 64 * half
                    for b in range(4):  # idle GpSimd queue; contiguous rows
                        nc.gpsimd.dma_start(
                            kT_both[p0:p0 + 64, :, 2 * b * 128:2 * b * 128 + 256],
                            k_g[l][b, :, :, :])
                for b in range(4):
                    nc.sync.dma_start(v65[:, 2 * b, :, 0:HD],
                                      v_g[l][b, :, 0, :, :])
                    nc.sync.dma_start(v65[:, 2 * b + 1, :, 0:HD],
                                      v_g[l][b, :, 1, :, :])

                if dbg and l == 0:
                    nc.sync.dma_start(dbg_t["dbg_qT"][:], qT[:])
                    nc.sync.dma_start(dbg_t["dbg_kT"][:], kT_both[:])
                    nc.sync.dma_start(dbg_t["dbg_v65"][:], v65[:])
                # ---- attention: rt0 sees key blocks 0-3, rt1 sees 0-7 ----
                # scores go through PSUM with the additive causal mask
                # accumulated by an identity matmul; one exp per (g,kt).
                oT = ap.tile([128, 8, R], BF16, tag="oT")
                with (
                    tc.tile_pool(name=f"ps_sc{l}", bufs=2, space="PSUM") as psc,
                    tc.tile_pool(name=f"ps_ot{l}", bufs=2, space="PSUM") as pso,
                ):
                    for g in range(KV):
                        # po[rt] accumulates [hd+1, j, i, 128] for the rt
                        # row-half; bufs=2 so g+1 scores overlap g's tail
                        po = [pso.tile([HD + 1, 2, 2, 128], F32,
                                       name=f"po{l}{g}{rt}", tag=f"ot{rt}",
                                       bufs=2)
                              for rt in range(2)]
                        for kt in range(NKT):
                            both = (kt % 2 == 0)
                            wsl = slice(0, R) if both else slice(128, R)
                            sc = psc.tile([128, 2, 2, R], F32, tag="sc", bufs=2)
                            for j in range(2):
                                nc.tensor.matmul(
                                    sc[:, j, :, wsl],
                                    kT_both[64 * j:64 * j + 64, g,
                                            kt * 128:(kt + 1) * 128],
                                    qT[64 * j:64 * j + 64, 2 * g:2 * g + 2, wsl],
                                    start=True, stop=True)
                            # accumulate the additive mask into the half this
                            # kt's mask governs: kt 0-3 -> rt0, kt 4-7 -> rt1
                            msl = slice(0, 128) if both else slice(128, R)
                            nc.tensor.matmul(
                                sc[:, :, :, msl], ident[:],
                                mask_sb[:, kt, :].unsqueeze(1).unsqueeze(1)
                                .broadcast_to((128, 2, 2, 128)),
                                start=False, stop=True, skip_group_check=True)
                            er = ap.tile([128, 2, 2, R], BF16, tag="expraw", bufs=3)
                            nc.scalar.activation(er[:, :, :, wsl], sc[:, :, :, wsl],
                                                 AF.Exp,
                                                 scale=float(1.0 / np.sqrt(HD)))
                            if dbg and l == 0 and g == 0:
                                nc.sync.dma_start(dbg_t["dbg_er"][:, kt], er[:])
                            for j in range(2):
                                if both:  # rt0 half: masked (even slots)
                                    nc.tensor.matmul(
                                        po[0][:, j], v65[:, kt, g, :],
                                        er[:, j, :, 0:128],
                                        start=(kt == 0), stop=(kt == 6),
                                        skip_group_check=True)
                                    # rt1 half: fully visible here
                                    nc.tensor.matmul(
                                        po[1][:, j], v65[:, kt, g, :],
                                        er[:, j, :, 128:R],
                                        start=(kt == 0), stop=False,
                                        skip_group_check=True)
                                else:     # rt1 half: masked (slots 4-7)
                                    nc.tensor.matmul(
                                        po[1][:, j], v65[:, kt, g, :],
                                        er[:, j, :, 128:R],
                                        start=False, stop=(kt == NKT - 1),
                                        skip_group_check=True)
                        # softmax denominators: row HD of po holds sum(exp);
                        # flat layout [_, rt*4 + j*2 + i, 128]
                        den4 = ap.tile([1, 8, 128], F32, tag="den4", bufs=1)
                        for rt in range(2):
                            nc.vector.tensor_copy(den4[:, rt * 4:rt * 4 + 4, :],
                                                  po[rt][HD:HD + 1, :, :, :])
                        bcs = ap.tile([64, 8, 128], F32, tag="bcs", bufs=1)
                        nc.gpsimd.partition_broadcast(bcs[:], den4[:])
                        rec = ap.tile([64, 8, 128], F32, tag="rec", bufs=2)
                        nc.vector.reciprocal_approx_fast(rec[:], bcs[:])
                        for s in range(4):
                            hq = 4 * g + s
                            j, i = s % 2, s // 2
                            if hq % 2 == 0:
                                for rt in range(2):
                                    nc.vector.tensor_tensor(
                                        oT[0:64, hq // 2, rt * 128:(rt + 1) * 128],
                                        po[rt][0:HD, j, i, :],
                                        rec[:, rt * 4 + j * 2 + i, :],
                                        mybir.AluOpType.mult)
                            else:
                                otmp = ap.tile([64, R], BF16, tag="otmp", bufs=2)
                                for rt in range(2):
                                    nc.vector.tensor_tensor(
                                        otmp[:, rt * 128:(rt + 1) * 128],
                                        po[rt][0:HD, j, i, :],
                                        rec[:, rt * 4 + j * 2 + i, :],
                                        mybir.AluOpType.mult)
                                nc.sync.dma_start(oT[64:128, hq // 2, :], otmp[:])

                if dbg and l == 0:
                    nc.sync.dma_start(dbg_t["dbg_oT"][:], oT[:])
                # ---- Wo + residual ----
                wob = ap.tile([128, DKT, D], BF16, name=f"wo{l}", tag="wbig",
                              bufs=3)
                nc.sync.dma_start(wob[:], woT[l])
                with tc.tile_pool(name=f"ps_wo{l}", bufs=1, space="PSUM") as pwo:
                    pswo = pwo.tile([128, RT, D], F32, tag="pswo")  # 4 banks
                    for rt in range(RT):  # rt-outer: h[rt0] residual lands
                        for kt in range(DKT):  # while rt1's Wo still runs
                            for nch in range(2):
                                nc.tensor.matmul(
                                    pswo[:, rt, nch * 512:(nch + 1) * 512],
                                    oT[:, kt, rt * 128:(rt + 1) * 128],
                                    wob[:, kt, nch * 512:(nch + 1) * 512],
                                    start=(kt == 0), stop=(kt == DKT - 1))
                        nc.vector.tensor_tensor(h[:, rt, :], h[:, rt, :],
                                                pswo[:, rt, :], mybir.AluOpType.add)

                # ---- MLP: transposed intermediate mT[f, r] ----
                r3, _ = rms_scales(h, f"n3_{l}")
                fnb = ap.tile([128, RT, D], BF16, tag="qkb")
                for rt in range(RT):
                    nc.vector.tensor_scalar(fnb[:, rt, :], h[:, rt, :],
                                            r3[:, rt : rt + 1], None,
                                            mybir.AluOpType.mult)
                ptp2 = tc.alloc_tile_pool(name=f"ps_tp2{l}", bufs=2, space="PSUM")
                fnT = ap.tile([128, DKT, R], BF16, tag="xT")
                for kt in range(DKT):
                    for rt in range(RT):
                        transpose_to(fnT[:, kt, rt * 128:(rt + 1) * 128],
                                     fnb[:, rt, kt * 128:(kt + 1) * 128],
                                     ptp2, f"fn{l}")

                mT = ap.tile([128, 32, R], BF16, tag="mT")
                with tc.tile_pool(name=f"ps_ff{l}", bufs=1, space="PSUM") as pff:
                    for ch in range(4):
                        wgb = ap.tile([128, DKT, 1024], BF16,
                                      name=f"wg{l}{ch}", tag="wbig", bufs=3)
                        wub = ap.tile([128, DKT, 1024], BF16,
                                      name=f"wu{l}{ch}", tag="wbig", bufs=3)
                        nc.sync.dma_start(wgb[:], wgT[l][:, ch])
                        nc.sync.dma_start(wub[:], wuT[l][:, ch])
                        for ft in range(8):
                            gi = ch * 8 + ft
                            psg = pff.tile([128, R], F32, tag="mmgu", bufs=4)
                            for kt in range(DKT):
                                nc.tensor.matmul(
                                    psg[:],
                                    wgb[:, kt, ft * 128:(ft + 1) * 128],
                                    fnT[:, kt, :],
                                    start=(kt == 0), stop=(kt == DKT - 1))
                            psu = pff.tile([128, R], F32, tag="mmgu", bufs=4)
                            for kt in range(DKT):
                                nc.tensor.matmul(
                                    psu[:],
                                    wub[:, kt, ft * 128:(ft + 1) * 128],
                                    fnT[:, kt, :],
                                    start=(kt == 0), stop=(kt == DKT - 1))
                            gsT = ap.tile([128, R], BF16, tag="gsT", bufs=3)
                            nc.scalar.activation(gsT[:], psg[:], AF.Silu)
                            nc.vector.tensor_tensor(mT[:, gi, :], psu[:], gsT[:],
                                                    mybir.AluOpType.mult)

                # Wd: h += mT.T @ wdT  (contract f over 32 tiles)
                with tc.tile_pool(name=f"ps_wd{l}", bufs=1, space="PSUM") as pwd:
                    pswd = pwd.tile([128, RT, D], F32, tag="pswd")  # 4 banks
                    for blk in range(4):
                        wdb = ap.tile([128, DKT, D], BF16,
                                      name=f"wd{l}{blk}", tag="wbig", bufs=3)
                        nc.sync.dma_start(wdb[:], wdT[l][:, blk])
                        for t in range(DKT):
                            ft = blk * 8 + t
                            for rt in range(RT):
                                for nch in range(2):
                                    nc.tensor.matmul(
                                        pswd[:, rt, nch * 512:(nch + 1) * 512],
                                        mT[:, ft, rt * 128:(rt + 1) * 128],
                                        wdb[:, t, nch * 512:(nch + 1) * 512],
                                        start=(ft == 0), stop=(ft == 31))
                    for rt in range(RT):
                        nc.vector.tensor_tensor(h[:, rt, :], h[:, rt, :],
                                                pswd[:, rt, :],
                                                mybir.AluOpType.add)
                if dbg:
                    nc.sync.dma_start(dbg_t[f"dbg_h{l + 1}"][:], h[:])
                ptp2.release()

            # ================= final norm + patch + logits =================
            r4, _ = rms_scales(h, "fin")
            hfb = ap.tile([128, RT, D], BF16, tag="hnb")
            for rt in range(RT):
                nc.vector.tensor_scalar(hfb[:, rt, :], h[:, rt, :],
                                        r4[:, rt : rt + 1], None,
                                        mybir.AluOpType.mult)
            hfT = ap.tile([128, DKT, R], BF16, tag="qT")
            with tc.tile_pool(name="ps_tph", bufs=2, space="PSUM") as pth:
                for rt in range(RT):
                    for kt in range(DKT):
                        transpose_to(hfT[:, kt, rt * 128:(rt + 1) * 128],
                                     hfb[:, rt, kt * 128:(kt + 1) * 128],
                                     pth, "hf")
            pt_sb = wp.tile([128, DKT, P], BF16, tag="pt_sb", bufs=1)
            for kt in range(DKT):
                nc.sync.dma_start(pt_sb[:, kt, :], patchT[kt * 128:(kt + 1) * 128, :])
            ec0 = ap.tile([128, VSH], BF16, tag="ec0")
            ec1 = ap.tile([128, VSH], BF16, tag="ec1")
            nc.sync.dma_start(ec0[:], ecT[0:128, :])
            nc.sync.dma_start(ec1[:], ecT[128:256, :])
            ec = [ec0, ec1]
            hp_stage = ap.tile([128, 2, R], BF16, tag="hp_stage")
            # patch GEMM + gather per 128-row half: the second gather rides
            # under the first half's logits GEMM
            with tc.tile_pool(name="ps_hp", bufs=2, space="PSUM") as php:
                for s in range(2):
                    for mt in range(2):
                        psp = php.tile([128, 128], F32, tag="mm")
                        for kt in range(DKT):
                            nc.tensor.matmul(
                                psp[:],
                                pt_sb[:, kt, mt * 128:(mt + 1) * 128],
                                hfT[:, kt, s * 128:(s + 1) * 128],
                                start=(kt == 0), stop=(kt == DKT - 1))
                        nc.scalar.copy(hp_stage[:, mt, s * 128:(s + 1) * 128],
                                       psp[:])
                    if dbg and s == 1:
                        nc.sync.dma_start(dbg_t["dbg_hp"][:], hp_stage[:])
                    nc.sync.dma_start(hp_in[s][:], hp_stage[:, :, s * 128:(s + 1) * 128])
                    nc.gpsimd.collective_compute(
                        "AllGather", mybir.AluOpType.bypass,
                        replica_groups=GROUPS_ALL,
                        ins=[hp_in[s][:].opt()], outs=[hp_g[s][:].opt()])
            hpT = ap.tile([128, 16, R], BF16, tag="mT")
            for s in range(2):
                for rbk in range(8):
                    nc.sync.dma_start(
                        hpT[:, 2 * rbk:2 * rbk + 2, s * 128:(s + 1) * 128],
                        hp_g[s][rbk])
            with tc.tile_pool(name="ps_lg", bufs=2, space="PSUM") as plg:
                for s in range(2):
                    for rbk in range(8):
                        orow = ((rbk // 4) * 8
                                + (rbk % 4 if s == 0 else 7 - rbk % 4)) * 128
                        for half in range(2):
                            plt = plg.tile([128, 4, 512], F32, tag="lg")
                            for kt in range(2):
                                for nq in range(4):
                                    nc.tensor.matmul(
                                        plt[:, nq, 0:500],
                                        hpT[:, 2 * rbk + kt, s * 128:(s + 1) * 128],
                                        ec[kt][:, half * 2000 + nq * 500:
                                               half * 2000 + (nq + 1) * 500],
                                        start=(kt == 0), stop=(kt == 1))
                            lg_sb = ap.tile([128, 2000], BF16, tag="lg_sb", bufs=2)
                            for nq in range(4):
                                if half == 0:
                                    nc.scalar.copy(lg_sb[:, nq * 500:(nq + 1) * 500],
                                                   plt[:, nq, 0:500])
                                else:
                                    nc.vector.tensor_copy(
                                        lg_sb[:, nq * 500:(nq + 1) * 500],
                                        plt[:, nq, 0:500])
                            nc.sync.dma_start(
                                out[orow:orow + 128,
                                    half * 2000:(half + 1) * 2000], lg_sb[:])

    nc.compile()
    _cache[key] = nc
    return nc


def _prep_inputs(x, z0, E, W_embed_up, W_z0, patch_W, final_norm_w,
                 norm1_w, q_norm_w, k_norm_w, norm2_w,
                 Wq, Wk, Wv, Wo, Wg, Wu, Wd):
    bf = ml_dtypes.bfloat16
    f32 = np.float32
    E = np.asarray(E, f32)
    x = np.asarray(x).astype(np.int64).reshape(B * T)

    zproj = np.asarray(z0, f32) @ np.asarray(W_z0, f32).T  # (B, D)

    def t(a):
        return np.ascontiguousarray(np.asarray(a, f32).T).astype(bf)

    def pk(wt):
        # [D_in, C] -> [128, D_in//128, C] with p fastest over D_in
        din, c = wt.shape
        return np.ascontiguousarray(
            wt.reshape(din // 128, 128, c).transpose(1, 0, 2))

    def pkch(wt, nch):
        # [D_in, C] -> [128, nch, D_in//128, C//nch]
        din, c = wt.shape
        cw = c // nch
        return np.ascontiguousarray(
            wt.reshape(din // 128, 128, nch, cw).transpose(1, 2, 0, 3))

    def pkblk(wt, nblk):
        # [D_in, C] -> [128, nblk, D_in//(128*nblk), C]
        din, c = wt.shape
        t_ = din // (128 * nblk)
        return np.ascontiguousarray(
            wt.reshape(nblk, t_, 128, c).transpose(2, 0, 1, 3))

    wqTn = np.stack([pk(t(np.asarray(Wq[l], f32) * np.asarray(q_norm_w[l], f32)[None, :]))
                     for l in range(L)])
    wkTn = np.stack([pk(t(np.asarray(Wk[l], f32) * np.asarray(k_norm_w[l], f32)[None, :]))
                     for l in range(L)])
    wvTn = np.stack([pk(t(Wv[l])) for l in range(L)])
    woTn = np.stack([pk(t(Wo[l])) for l in range(L)])
    wgTn = np.stack([pkch(t(np.asarray(Wg[l], f32) * np.asarray(norm2_w[l], f32)[None, :]), 4)
                     for l in range(L)])
    wuTn = np.stack([pkch(t(np.asarray(Wu[l], f32) * np.asarray(norm2_w[l], f32)[None, :]), 4)
                     for l in range(L)])
    wdTn = np.stack([pkblk(t(Wd[l]), 4) for l in range(L)])
    patchTn = t(np.asarray(patch_W, f32) * np.asarray(final_norm_w, f32)[None, :])
    n1rep = np.stack([np.broadcast_to(np.asarray(norm1_w[l], f32), (128, D)).copy()
                      for l in range(L)])

    wembT = np.asarray(W_embed_up, f32).T  # (P, D)
    in_maps = []
    kk = np.arange(128)
    rr = np.arange(128)
    for c in range(N_CORES):
        q = c % 4
        b0, b1 = q, 7 - q          # row blocks for rt0, rt1
        rows = np.concatenate([
            x[(c // 4) * T + b0 * 128:(c // 4) * T + b0 * 128 + 128],
            x[(c // 4) * T + b1 * 128:(c // 4) * T + b1 * 128 + 128]])
        ex = E[rows]                       # (R, P)
        ex_augT = np.zeros((384, R), f32)
        ex_augT[:P, :] = ex.T
        ex_augT[P, :] = 1.0
        w_emb = np.zeros((384, D), f32)
        w_emb[:P, :] = wembT
        w_emb[P, :] = zproj[c // 4]

        # additive mask slots: 0-3 gate rt0 (block b0) vs key blocks 0-3;
        # 4-7 gate rt1 (block b1) vs key blocks 4-7. 0 = visible, -1e9 = hidden
        mask2 = np.zeros((NKT, 128, 128), f32)
        for b in range(4):
            mask2[2 * b] = np.where(
                (b * 128 + kk[:, None]) <= (b0 * 128 + rr[None, :]), 0.0, -1e9)
            gb = 7 - b
            mask2[2 * b + 1] = np.where(
                (gb * 128 + kk[:, None]) <= (b1 * 128 + rr[None, :]), 0.0, -1e9)
        mask2 = mask2.astype(bf)

        ecTn = np.ascontiguousarray(E[c * VSH:(c + 1) * VSH].T).astype(bf)

        in_maps.append({
            "ex_augT": ex_augT.astype(bf), "w_emb": w_emb.astype(bf),
            "wqT": wqTn, "wkT": wkTn, "wvT": wvTn, "woT": woTn,
            "wgT": wgTn, "wuT": wuTn, "wdT": wdTn,
            "n1rep": n1rep, "mask2": mask2, "patchT": patchTn, "ecT": ecTn,
        })
    return in_maps


last_exec_ns = None


def kernel(**inputs) -> np.ndarray:
    global last_exec_ns
    in_maps = _prep_inputs(**inputs)
    nc = build()
    trace = bool(int(os.environ.get("TRN_PROFILE", "0")))
    kw = {}
    if trace:
        try:
            import prof_shim
            prof_shim.install()
            kw = dict(trace=True, tmpdir=os.environ.get("TRN_TRACE_DIR", None))
        except Exception:
            kw = {}
    res = run_bass_kernel_spmd(nc, in_maps, CORE_IDS, **kw)
    last_exec_ns = res.exec_time_ns
    # logits per core: [B*T, VSH] already in global row order; concat vocab
    parts = [np.asarray(res.results[c]["logits"]).astype(np.float32)
             for c in range(N_CORES)]
    return np.concatenate(parts, axis=1).reshape(B, T, V)
